# revision 5
# baseline (speedup 1.0000x reference)
"""MoE (noisy top-2 routing, dense expert stack) on 8 Trainium2 NeuronCores.

Strategy: load-balanced expert-parallel with host-side routing as the
sharding step. The host computes the noisy gating in fp64 (bit-robust
reproduction of the reference's fp32 top-2 selection) and also the top-2
softmax weight of each (token, expert) pair, so the device runs nothing but
the expert FFNs at the PE-array roofline.

Work is balanced in 128-token chunks: expert e needs ceil(count_e/128)
chunks; every core is primary for one expert (16 chunks = 2048 tokens) and
takes at most one 128-token spill chunk from an overloaded expert as a
secondary weight set, so all 8 cores run the same SPMD program over
2176 slots (the chunk-granular optimum of max-load balancing) instead of
padding every core to the most-loaded expert (2304).

All matmul operands are bf16 (same 1 cycle/row PE rate as fp32r, but full
rate at any tile width and half the DMA/SBUF footprint, which lets both
experts' W1 and W2 stay SBUF-resident). PSUM accumulation stays fp32; the
weighted rows are stored fp32 and scatter-added on the host as the
unsharding step.
"""

import sys

sys.path.insert(0, "/opt/trn_rl_repo")

import math

import ml_dtypes
import numpy as np

import concourse.mybir as mybir
import concourse.tile as tile
from concourse import bacc
from concourse.bass_utils import run_bass_kernel_spmd

N_CORES = 8
N, D, H, E = 8192, 1024, 2048, 8
P = 128
KD = D // P                 # 8  k-chunks over D
KH = H // P                 # 16 h-chunks
ACH = 16                    # primary-expert chunks per core
SLOTS = (ACH + 1) * P       # 2176 = 16 primary chunks + 1 spill chunk

F32 = mybir.dt.float32
BF16 = mybir.dt.bfloat16
ALU = mybir.AluOpType
ACT_F = mybir.ActivationFunctionType
BF = ml_dtypes.bfloat16


def _build(slots, repeat=1):
    """SPMD program: FFN of weight set A over 4x512 tokens, then weight set
    B over the 128-token spill chunk. Per-token combine weights arrive
    precomputed in wcd; gating runs entirely on the host."""
    assert slots == SLOTS
    widths = [(512, "A"), (512, "A"), (512, "A"), (512, "A"), (P, "B")]

    nc = bacc.Bacc(None, target_bir_lowering=False, debug=False)

    xTg = nc.dram_tensor("xTg", [D, slots], BF16, kind="ExternalInput")
    wcd = nc.dram_tensor("wcd", [P, slots // P], F32, kind="ExternalInput")
    dram_w = {
        s: (
            nc.dram_tensor(f"W1{s}", [D, H], BF16, kind="ExternalInput"),
            nc.dram_tensor(f"b1{s}", [H], F32, kind="ExternalInput"),
            nc.dram_tensor(f"W2{s}", [H, D], BF16, kind="ExternalInput"),
            nc.dram_tensor(f"b2{s}", [D], F32, kind="ExternalInput"),
        )
        for s in ("A", "B")
    }
    yc = nc.dram_tensor("yc", [slots, D], F32, kind="ExternalOutput")

    with tile.TileContext(nc) as tc:
        with (
            tc.tile_pool(name="persist", bufs=1) as persist,
            tc.tile_pool(name="xs", bufs=2) as xs,
            tc.tile_pool(name="yws", bufs=3) as yws,
            tc.tile_pool(name="ph", bufs=3, space="PSUM") as ph,
            tc.tile_pool(name="py", bufs=4, space="PSUM") as py,
        ):
            # ---- persistent tiles: both experts' weights stay resident ----
            W1sb, W2sb, b1sb, b2sb = {}, {}, {}, {}
            for s in ("A", "B"):
                W1d, b1d, W2d, b2d = dram_w[s]
                W1sb[s] = persist.tile([P, KD, H], BF16, name=f"W1sb{s}")
                # quarter-sliced so tile-0's first matmuls only wait on the
                # first quarter at kernel start
                for q in range(4):
                    qs = slice(q * (H // 4), (q + 1) * (H // 4))
                    nc.sync.dma_start(
                        W1sb[s][:, :, qs],
                        W1d[:, qs].rearrange("(kd p) h -> p kd h", p=P),
                    )
                W2sb[s] = persist.tile([P, KH, D], BF16, name=f"W2sb{s}")
                nc.sync.dma_start(
                    W2sb[s][:], W2d.rearrange("(kh p) d -> p kh d", p=P)
                )
                b1sb[s] = persist.tile([P, KH], F32, name=f"b1sb{s}")
                nc.sync.dma_start(
                    b1sb[s][:], b1d.rearrange("(m p) -> p m", p=P)
                )
                b2sb[s] = persist.tile([P, D], F32, name=f"b2sb{s}")
                nc.sync.dma_start(b2sb[s][:], b2d[None, :].to_broadcast((P, D)))
            wc = persist.tile([P, slots // P], F32)
            nc.sync.dma_start(wc[:], wcd[:, :])

            # hT split into 4 tiles so L1-eviction -> L2 and L2-read ->
            # next-tile-L1 dependencies are tracked at quarter granularity
            hts = [persist.tile([P, 4, 512], BF16, name=f"hT{i}") for i in range(4)]

            for _rep in range(repeat):
                base = 0
                for TW, s in widths:
                    nch = TW // P
                    ss = slice(base, base + TW)
                    xg = xs.tile([P, KD, 512], BF16, tag="xg")
                    nc.sync.dma_start(
                        xg[:, :, :TW],
                        xTg[:, ss].rearrange("(kd p) t -> p kd t", p=P),
                    )

                    # layer 1: hT = relu(W1^T-chunk @ x + b1), h on partitions
                    for m in range(KH):
                        h_ps = ph.tile([P, 512], F32, tag="hps")
                        for kd in range(KD):
                            nc.tensor.matmul(
                                h_ps[:, :TW],
                                W1sb[s][:, kd, m * P : (m + 1) * P],
                                xg[:, kd, :TW],
                                start=(kd == 0),
                                stop=(kd == KD - 1),
                            )
                        nc.scalar.activation(
                            hts[m // 4][:, m % 4, :TW],
                            h_ps[:, :TW],
                            ACT_F.Relu,
                            bias=b1sb[s][:, m : m + 1],
                        )

                    # layer 2 + bias + combine weight + store, per 128-token
                    # chunk; epilogue of chunk c overlaps matmuls of c+1
                    for c4 in range(nch):
                        cs = slice(c4 * P, (c4 + 1) * P)
                        ch = base // P + c4
                        for nh in range(2):
                            ns = slice(nh * 512, (nh + 1) * 512)
                            y_ps = py.tile([P, 512], F32, tag="yps")
                            for kh in range(KH):
                                nc.tensor.matmul(
                                    y_ps[:],
                                    hts[kh // 4][:, kh % 4, cs],
                                    W2sb[s][:, kh, ns],
                                    start=(kh == 0),
                                    stop=(kh == KH - 1),
                                )
                            yw = yws.tile([P, 512], F32, tag="yw")
                            nc.vector.tensor_tensor(
                                yw[:], y_ps[:], b2sb[s][:, ns], ALU.add
                            )
                            nc.vector.tensor_scalar(
                                yw[:], yw[:], wc[:, ch : ch + 1], None, ALU.mult
                            )
                            nc.sync.dma_start(
                                yc[base + c4 * P : base + (c4 + 1) * P, ns],
                                yw[:],
                            )
                    base += TW

    nc.compile()
    return nc


_NC_CACHE = {}


def _get_nc(slots, repeat=1):
    key = (slots, repeat)
    if key not in _NC_CACHE:
        _NC_CACHE[key] = _build(slots, repeat)
    return _NC_CACHE[key]


def prepare(x, W1, b1, W2, b2, Wg, bg, noise, **_ignored):
    """Host-side routing/sharding: fp64 noisy top-2 + softmax weights,
    chunk-balanced per-core token lists, per-core input maps, and the
    scatter-add spec for unsharding."""
    x = np.ascontiguousarray(np.asarray(x, dtype=np.float32))
    noise = np.asarray(noise, dtype=np.float32)
    W1 = np.asarray(W1, dtype=np.float32)
    b1 = np.asarray(b1, dtype=np.float32)
    W2 = np.asarray(W2, dtype=np.float32)
    b2 = np.asarray(b2, dtype=np.float32)
    Wg = np.asarray(Wg, dtype=np.float32)
    bg = np.asarray(bg, dtype=np.float32)

    noisy = (
        x.astype(np.float64) @ Wg.astype(np.float64)
        + bg.astype(np.float64)
        + 0.1 * noise.astype(np.float64)
    )
    top2 = np.argsort(-noisy, axis=1)[:, :2]
    v = np.take_along_axis(noisy, top2, axis=1)
    w0 = 1.0 / (1.0 + np.exp(v[:, 1] - v[:, 0]))   # softmax over the top-2
    wmat = np.zeros((N, E), dtype=np.float32)
    wmat[np.arange(N), top2[:, 0]] = w0.astype(np.float32)
    wmat[np.arange(N), top2[:, 1]] = (1.0 - w0).astype(np.float32)

    tok_lists = [np.nonzero((top2 == e).any(axis=1))[0] for e in range(E)]
    n_chunks = [max(ACH, math.ceil(len(t) / P)) for t in tok_lists]
    spill = [
        (e, k) for e in range(E) for k in range(ACH, n_chunks[e])
    ]
    assert len(spill) <= N_CORES, f"spill chunks {len(spill)} > cores"

    x_bf = x.astype(BF)
    W1_bf = W1.astype(BF)
    W2_bf = W2.astype(BF)

    def padded(e, lo, hi):
        """Token ids / weights for padded slot range [lo, hi) of expert e;
        pad slots get token 0 with weight 0."""
        toks = tok_lists[e]
        cnt = min(max(len(toks) - lo, 0), hi - lo)
        idx = np.zeros(hi - lo, dtype=np.int64)
        idx[:cnt] = toks[lo : lo + cnt]
        wts = np.zeros(hi - lo, dtype=np.float32)
        wts[:cnt] = wmat[idx[:cnt], e]
        return idx, wts, toks[lo : lo + cnt]

    in_maps = []
    gathers = []
    for c in range(N_CORES):
        idxA, wA, toksA = padded(c, 0, ACH * P)
        if c < len(spill):
            f, k = spill[c]
            idxB, wB, toksB = padded(f, k * P, (k + 1) * P)
        else:
            f = c
            idxB = np.zeros(P, dtype=np.int64)
            wB = np.zeros(P, dtype=np.float32)
            toksB = np.zeros(0, dtype=np.int64)
        idx = np.concatenate([idxA, idxB])
        wts = np.concatenate([wA, wB])
        in_maps.append(
            {
                "xTg": np.ascontiguousarray(x_bf[idx].T),
                "wcd": np.ascontiguousarray(wts.reshape(-1, P).T),
                "W1A": W1_bf[c],
                "b1A": b1[c],
                "W2A": W2_bf[c],
                "b2A": b2[c],
                "W1B": W1_bf[f],
                "b1B": b1[f],
                "W2B": W2_bf[f],
                "b2B": b2[f],
            }
        )
        gathers.append((toksA, toksB))
    return in_maps, gathers, SLOTS


def combine(results, gathers):
    """Unshard: scatter-add each core's pre-weighted rows into the output."""
    out = np.zeros((N, D), dtype=np.float32)
    for c in range(N_CORES):
        toksA, toksB = gathers[c]
        ycv = results[c]["yc"]
        out[toksA] += ycv[: len(toksA)]
        if len(toksB):
            out[toksB] += ycv[ACH * P : ACH * P + len(toksB)]
    return out


def kernel(x, W1, b1, W2, b2, Wg, bg, noise, **_ignored):
    in_maps, gathers, slots = prepare(x, W1, b1, W2, b2, Wg, bg, noise)
    nc = _get_nc(slots)
    res = run_bass_kernel_spmd(nc, in_maps, core_ids=list(range(N_CORES)))
    return combine(res.results, gathers)


# revision 19
# speedup vs baseline: 1.0768x; 1.0768x over previous
"""MoE (noisy top-2 routing, dense expert stack) on 8 Trainium2 NeuronCores.

Strategy: load-balanced expert-parallel with host-side routing as the
sharding step. The host computes the noisy gating in fp64 (bit-robust
reproduction of the reference's fp32 top-2 selection) and also the top-2
softmax weight of each (token, expert) pair, so the device runs nothing but
the expert FFNs at the PE-array roofline.

Work is balanced in 128-token chunks: expert e needs ceil(count_e/128)
chunks; every core is primary for one expert (16 chunks = 2048 tokens) and
takes at most one 128-token spill chunk from an overloaded expert as a
secondary weight set, so all 8 cores run the same SPMD program over
2176 slots (the chunk-granular optimum of max-load balancing) instead of
padding every core to the most-loaded expert (2304).

All matmul operands are bf16 (same 1 cycle/row PE rate as fp32r, but full
rate at any tile width and half the DMA/SBUF footprint, which lets both
experts' W1 and W2 stay SBUF-resident). PSUM accumulation stays fp32; the
weighted rows are stored fp32 and scatter-added on the host as the
unsharding step.
"""

import sys

sys.path.insert(0, "/opt/trn_rl_repo")

import math

import ml_dtypes
import numpy as np

import concourse.mybir as mybir
import concourse.tile as tile
from concourse import bacc
from concourse.bass_utils import run_bass_kernel_spmd

N_CORES = 8
N, D, H, E = 8192, 1024, 2048, 8
P = 128
KD = D // P                 # 8  k-chunks over D
KH = H // P                 # 16 h-chunks
ACH = 16                    # primary-expert chunks per core
SLOTS = (ACH + 1) * P       # 2176 = 16 primary chunks + 1 spill chunk

F32 = mybir.dt.float32
BF16 = mybir.dt.bfloat16
ALU = mybir.AluOpType
ACT_F = mybir.ActivationFunctionType
BF = ml_dtypes.bfloat16


def _build(slots, repeat=1):
    """SPMD program: FFN of weight set A over 4x512 tokens, then weight set
    B over the 128-token spill chunk. Per-token combine weights arrive
    precomputed in wcd; gating runs entirely on the host."""
    assert slots == SLOTS
    widths = [(512, "A"), (512, "A"), (512, "A"), (512, "A"), (P, "B")]

    nc = bacc.Bacc(None, target_bir_lowering=False, debug=False)

    xTg = nc.dram_tensor("xTg", [D, slots], BF16, kind="ExternalInput")
    wcd = nc.dram_tensor("wcd", [P, slots // P], F32, kind="ExternalInput")
    dram_w = {
        s: (
            nc.dram_tensor(f"W1{s}", [D, H], BF16, kind="ExternalInput"),
            nc.dram_tensor(f"b1{s}", [H], F32, kind="ExternalInput"),
            nc.dram_tensor(f"W2{s}", [H, D], BF16, kind="ExternalInput"),
            nc.dram_tensor(f"b2{s}", [D], F32, kind="ExternalInput"),
        )
        for s in ("A", "B")
    }
    yc = nc.dram_tensor("yc", [slots, D], F32, kind="ExternalOutput")

    with tile.TileContext(nc) as tc:
        with (
            tc.tile_pool(name="persist", bufs=1) as persist,
            tc.tile_pool(name="xs", bufs=2) as xs,
            tc.tile_pool(name="yws", bufs=3) as yws,
            tc.tile_pool(name="ph", bufs=3, space="PSUM") as ph,
            tc.tile_pool(name="py", bufs=4, space="PSUM") as py,
        ):
            # ---- persistent tiles: both experts' weights stay resident ----
            # DMA issue order is the startup critical path: A's first W1
            # quarter, then tile-0's x prefetch, then the rest in first-use
            # order, so the PE starts ~7us in instead of waiting out all
            # 16MB of weight loads.
            W1sb, W2sb, b1sb, b2sb = {}, {}, {}, {}
            for s in ("A", "B"):
                W1d, b1d, W2d, b2d = dram_w[s]
                W1sb[s] = persist.tile([P, KD, H], BF16, name=f"W1sb{s}")
                W2sb[s] = persist.tile([P, KH, D], BF16, name=f"W2sb{s}")
                b1sb[s] = persist.tile([P, KH], F32, name=f"b1sb{s}")
                b2sb[s] = persist.tile([P, D], F32, name=f"b2sb{s}")
            wc = persist.tile([P, slots // P], F32)

            def load_w1_slice(s, q, nsl):
                qs = slice(q * (H // nsl), (q + 1) * (H // nsl))
                nc.sync.dma_start(
                    W1sb[s][:, :, qs],
                    dram_w[s][0][:, qs].rearrange("(kd p) h -> p kd h", p=P),
                )

            def load_b1(s):
                nc.sync.dma_start(
                    b1sb[s][:], dram_w[s][1].rearrange("(m p) -> p m", p=P)
                )

            def load_rest(s):
                nc.sync.dma_start(
                    W2sb[s][:], dram_w[s][2].rearrange("(kh p) d -> p kh d", p=P)
                )
                nc.sync.dma_start(
                    b2sb[s][:], dram_w[s][3][None, :].to_broadcast((P, D))
                )

            load_w1_slice("A", 0, 4)
            xg0 = xs.tile([P, KD, 512], BF16, tag="xg")
            nc.sync.dma_start(
                xg0[:], xTg[:, 0:512].rearrange("(kd p) t -> p kd t", p=P)
            )
            load_b1("A")   # 8KB, right behind xg0: m=0's activation needs it
            for q in range(1, 4):
                load_w1_slice("A", q, 4)
            load_rest("A")
            nc.sync.dma_start(wc[:], wcd[:, :])
            for q in range(4):
                load_w1_slice("B", q, 4)
            load_b1("B")
            load_rest("B")

            # hT split into 4 tiles so L1-eviction -> L2 and L2-read ->
            # next-tile-L1 dependencies are tracked at quarter granularity
            hts = [persist.tile([P, 4, 512], BF16, name=f"hT{i}") for i in range(4)]

            for _rep in range(repeat):
                base = 0
                for TW, s in widths:
                    nch = TW // P
                    ss = slice(base, base + TW)
                    if _rep == 0 and base == 0:
                        xg = xg0    # prefetched with the startup loads
                    else:
                        xg = xs.tile([P, KD, 512], BF16, tag="xg")
                        nc.sync.dma_start(
                            xg[:, :, :TW],
                            xTg[:, ss].rearrange("(kd p) t -> p kd t", p=P),
                        )

                    # layer 1: hT = relu(W1^T-chunk @ x + b1), h on partitions
                    for m in range(KH):
                        h_ps = ph.tile([P, 512], F32, tag="hps")
                        for kd in range(KD):
                            nc.tensor.matmul(
                                h_ps[:, :TW],
                                W1sb[s][:, kd, m * P : (m + 1) * P],
                                xg[:, kd, :TW],
                                start=(kd == 0),
                                stop=(kd == KD - 1),
                            )
                        nc.scalar.activation(
                            hts[m // 4][:, m % 4, :TW],
                            h_ps[:, :TW],
                            ACT_F.Relu,
                            bias=b1sb[s][:, m : m + 1],
                        )

                    # layer 2 + bias + combine weight + store, per 128-token
                    # chunk; epilogue of chunk c overlaps matmuls of c+1
                    for c4 in range(nch):
                        cs = slice(c4 * P, (c4 + 1) * P)
                        ch = base // P + c4
                        for nh in range(2):
                            ns = slice(nh * 512, (nh + 1) * 512)
                            y_ps = py.tile([P, 512], F32, tag="yps")
                            for kh in range(KH):
                                nc.tensor.matmul(
                                    y_ps[:],
                                    hts[kh // 4][:, kh % 4, cs],
                                    W2sb[s][:, kh, ns],
                                    start=(kh == 0),
                                    stop=(kh == KH - 1),
                                )
                            yw = yws.tile([P, 512], F32, tag="yw")
                            nc.vector.tensor_tensor(
                                yw[:], y_ps[:], b2sb[s][:, ns], ALU.add
                            )
                            nc.vector.tensor_scalar(
                                yw[:], yw[:], wc[:, ch : ch + 1], None, ALU.mult
                            )
                            nc.sync.dma_start(
                                yc[base + c4 * P : base + (c4 + 1) * P, ns],
                                yw[:],
                            )
                    base += TW

    nc.compile()
    return nc


_NC_CACHE = {}


def _get_nc(slots, repeat=1):
    key = (slots, repeat)
    if key not in _NC_CACHE:
        _NC_CACHE[key] = _build(slots, repeat)
    return _NC_CACHE[key]


def prepare(x, W1, b1, W2, b2, Wg, bg, noise, **_ignored):
    """Host-side routing/sharding: fp64 noisy top-2 + softmax weights,
    chunk-balanced per-core token lists, per-core input maps, and the
    scatter-add spec for unsharding."""
    x = np.ascontiguousarray(np.asarray(x, dtype=np.float32))
    noise = np.asarray(noise, dtype=np.float32)
    W1 = np.asarray(W1, dtype=np.float32)
    b1 = np.asarray(b1, dtype=np.float32)
    W2 = np.asarray(W2, dtype=np.float32)
    b2 = np.asarray(b2, dtype=np.float32)
    Wg = np.asarray(Wg, dtype=np.float32)
    bg = np.asarray(bg, dtype=np.float32)

    noisy = (
        x.astype(np.float64) @ Wg.astype(np.float64)
        + bg.astype(np.float64)
        + 0.1 * noise.astype(np.float64)
    )
    top2 = np.argsort(-noisy, axis=1)[:, :2]
    v = np.take_along_axis(noisy, top2, axis=1)
    w0 = 1.0 / (1.0 + np.exp(v[:, 1] - v[:, 0]))   # softmax over the top-2
    wmat = np.zeros((N, E), dtype=np.float32)
    wmat[np.arange(N), top2[:, 0]] = w0.astype(np.float32)
    wmat[np.arange(N), top2[:, 1]] = (1.0 - w0).astype(np.float32)

    tok_lists = [np.nonzero((top2 == e).any(axis=1))[0] for e in range(E)]
    n_chunks = [max(ACH, math.ceil(len(t) / P)) for t in tok_lists]
    spill = [
        (e, k) for e in range(E) for k in range(ACH, n_chunks[e])
    ]
    assert len(spill) <= N_CORES, f"spill chunks {len(spill)} > cores"

    x_bf = x.astype(BF)
    W1_bf = W1.astype(BF)
    W2_bf = W2.astype(BF)

    def padded(e, lo, hi):
        """Token ids / weights for padded slot range [lo, hi) of expert e;
        pad slots get token 0 with weight 0."""
        toks = tok_lists[e]
        cnt = min(max(len(toks) - lo, 0), hi - lo)
        idx = np.zeros(hi - lo, dtype=np.int64)
        idx[:cnt] = toks[lo : lo + cnt]
        wts = np.zeros(hi - lo, dtype=np.float32)
        wts[:cnt] = wmat[idx[:cnt], e]
        return idx, wts, toks[lo : lo + cnt]

    in_maps = []
    gathers = []
    for c in range(N_CORES):
        idxA, wA, toksA = padded(c, 0, ACH * P)
        if c < len(spill):
            f, k = spill[c]
            idxB, wB, toksB = padded(f, k * P, (k + 1) * P)
        else:
            f = c
            idxB = np.zeros(P, dtype=np.int64)
            wB = np.zeros(P, dtype=np.float32)
            toksB = np.zeros(0, dtype=np.int64)
        idx = np.concatenate([idxA, idxB])
        wts = np.concatenate([wA, wB])
        in_maps.append(
            {
                "xTg": np.ascontiguousarray(x_bf[idx].T),
                "wcd": np.ascontiguousarray(wts.reshape(-1, P).T),
                "W1A": W1_bf[c],
                "b1A": b1[c],
                "W2A": W2_bf[c],
                "b2A": b2[c],
                "W1B": W1_bf[f],
                "b1B": b1[f],
                "W2B": W2_bf[f],
                "b2B": b2[f],
            }
        )
        gathers.append((toksA, toksB))
    return in_maps, gathers, SLOTS


def combine(results, gathers):
    """Unshard: scatter-add each core's pre-weighted rows into the output."""
    out = np.zeros((N, D), dtype=np.float32)
    for c in range(N_CORES):
        toksA, toksB = gathers[c]
        ycv = results[c]["yc"]
        out[toksA] += ycv[: len(toksA)]
        if len(toksB):
            out[toksB] += ycv[ACH * P : ACH * P + len(toksB)]
    return out


def kernel(x, W1, b1, W2, b2, Wg, bg, noise, **_ignored):
    in_maps, gathers, slots = prepare(x, W1, b1, W2, b2, Wg, bg, noise)
    nc = _get_nc(slots)
    res = run_bass_kernel_spmd(nc, in_maps, core_ids=list(range(N_CORES)))
    return combine(res.results, gathers)
